# revision 27
# baseline (speedup 1.0000x reference)
"""Trainium2 Bass kernel for the Householder-chain problem.

Computes y = x @ Q.T where Q = M_0 @ M_1 @ ... @ M_{N-1} is a product of
N=514 Householder reflections M_i = I - 2 v_i v_i^T / (v_i^T v_i + eps)
over S=512 dims, and x is [65536, 512].

Math: since each M_i is symmetric, Q.T = M_{N-1} @ ... @ M_0 =: A, and the
product collapses via the compact-WY representation with natural column
order:  A = I - V T V^T  where V = [v_0 ... v_{N-1}] (S x N) and
T^{-1} = R = stril(V^T V) + diag((||v_i||^2 + eps)/2)   (lower triangular).

On device (replicated on each of 8 cores, since it is tiny):
  G = V^T V (f32r row-strip matmuls, all moving dims >= 256 so the PE runs
  at full rate); the five 128x128 diagonal blocks of R are inverted by
  Newton iteration X <- X(2I - R X) in bf16, run as two independent
  dependency chains (blocks 0-2 and 3-4) so engine latencies overlap.
  Off-diagonal blocks of X = R^{-1} come from a zero-padded full-row
  back-substitution in f32r (one wide accumulating matmul per block-row
  instead of per-(i,j) 128-wide matmuls); each back-substitution step
  feeds its row's terms of WT = (V T)^T into per-column PSUM accumulators
  immediately, then A = I - WT^T V (bf16 matmuls), cast to bf16.  N is zero-padded
  514 -> 640 with unit diagonal entries in R for the pad columns, which
  leaves A unchanged.

Main work: y = x @ A, data-parallel over the 65536 rows across 8 cores
(8192 rows/core).  It runs weight-stationary in bf16 producing y^T tiles:
out[c, r] = sum_k A[k-strip, c-strip]^T x^T[k-strip, r], with x^T uploaded
in bf16 (8 MB/core, fully resident in SBUF; the DMAs are issued first and
stream in behind the small v loads while the PE runs the prologue) and
y^T stored in bf16.  The host un-transposes and casts back to float32.
End-to-end relative error ~7e-3 (gate is 2e-2).
"""

from contextlib import ExitStack

import numpy as np
import ml_dtypes

import bass_rust
import concourse.bass as bass
import concourse.mybir as mybir
import concourse.tile as tile
from concourse.bass_utils import run_bass_kernel_spmd
from concourse.masks import make_identity, make_upper_triangular
from concourse.vector_clock import ScopedClock

FP = mybir.dt.float32
FPR = mybir.dt.float32r
BF = mybir.dt.bfloat16
AX = mybir.AxisListType
OP = mybir.AluOpType

S = 512           # feature dim
NV = 514          # number of householder vectors
NP = 640          # padded vector count (5 * 128)
NB = NP // 128    # 5 blocks
B = 65536         # batch rows
NCORES = 8
BPC = B // NCORES  # 8192 rows per core
EPS = 1e-16
NEWTON = 4        # bf16 Newton iterations (converged; see numerics note)
RW = 512          # main-loop r-block width (moving free dim)
RG = 1024         # store-group width (columns per output DMA)
NGA = 3           # Newton group a: blocks 0..2
WA, WB = NGA * 128, (NB - NGA) * 128


# ---------------------------------------------------------------------------
# walrus CTRL instructions accept at most 4 sem waits, and this Tile
# version puts the whole global-clock wait set on the single tail drain.
# Spread the waits over preceding SP nops (1 wait each, conservatively).
def _patched_drain_and_barrier(self, tick_clock, wait_clock):
    pre_nops = [self.nc.sync.nop() for _ in range(30)]
    drain_inst = self.nc.sync.drain()
    wait_clock.add_sem_waits(
        drain_inst.ins, ScopedClock({None: tick_clock.global_clock})
    )
    si = drain_inst.ins.sync_info
    waits = list(si.on_wait) if si is not None and si.on_wait else []
    if len(waits) > 1:
        assert len(waits) - 1 <= len(pre_nops), "too many drain waits"
        for nop, w in zip(pre_nops, waits[:-1]):
            nop.ins.sync_info = bass_rust.SyncInfo(on_wait=[w], on_update=[])
        upd = list(si.on_update) if si.on_update else []
        drain_inst.ins.sync_info = bass_rust.SyncInfo(
            on_wait=[waits[-1]], on_update=upd)

    self.nc.all_engine_barrier()
    assert self.sems is not None
    popped = self.nc._tile_sem_poison_stack.pop()
    assert popped is self._sem_poison
    # clear_and_free_semaphores, but issuing the dma_reset/sem_clear from
    # the Sync engine instead of GpSimd — gpsimd dispatch carries ~2us of
    # fixed overhead per op, which put ~4us of dead time in the kernel
    # tail barrier.
    sems = list(self.sems.allocated().values())
    if sems:
        sem_nums = [s.num if hasattr(s, "num") else s for s in sems]
        for r in bass.compact_to_ranges(sem_nums):
            assert self.nc._state.free_isdisjoint(r)
            self.nc.sync.drain(semaphore_range=r)
            self.nc.sync.sem_clear(r)
        self.nc._state.prepend_free_semaphores(sem_nums)
        for ps in self.nc._tile_sem_poison_stack:
            ps.update(sem_nums)
    self.nc.all_engine_barrier(sem_only=True)


tile.TileContext._drain_and_barrier = _patched_drain_and_barrier


def _split_excess_waits(nc, max_waits=1):
    """This walrus build accepts very few sem waits per instruction (a
    TensorTensor with 2 was rejected).  Hoist all but `max_waits` of each
    instruction's waits onto same-engine NOPs inserted right before it —
    engines execute in order, so semantics are unchanged."""
    idx = 0
    for fn in nc.m.functions:
        for bb in fn.blocks:
            new = []
            changed = False
            for inst in bb.instructions:
                si = inst.sync_info
                waits = list(si.on_wait) if si is not None and si.on_wait else []
                if len(waits) > max_waits:
                    changed = True
                    for w in waits[:-max_waits]:
                        idx += 1
                        nop = mybir.InstNoOp(
                            name=f"I-waitsplit-{idx}", engine=inst.engine)
                        nop.sync_info = bass_rust.SyncInfo(
                            on_wait=[w], on_update=[])
                        new.append(nop)
                    upd = list(si.on_update) if si.on_update else []
                    inst.sync_info = bass_rust.SyncInfo(
                        on_wait=waits[-max_waits:], on_update=upd)
                new.append(inst)
            if changed:
                bb.instructions = new
# ---------------------------------------------------------------------------


def _bs(b):
    return slice(b * 128, (b + 1) * 128)


def _emit_prologue(nc, vt_d, vnat_d, xt_d, xb, consts, work, psum):
    """Emit instructions computing A as 4 bf16 sbuf tiles [128(s), 512(c)].
    Issues all input DMAs first (v loads, then the big x^T loads)."""
    ptag = [0]

    def ptile(shape, name):  # rotating psum allocator (tags y0..y6)
        t = psum.tile(shape, FP, tag=f"y{ptag[0] % 7}", name=name)
        ptag[0] += 1
        return t

    # --- input DMAs first: v loads gate the prologue; x^T streams behind ---
    vtr = []
    for k in range(4):
        t = consts.tile([128, NP], FPR, tag=f"vt{k}", name=f"vt{k}")
        nc.sync.dma_start(out=t, in_=vt_d[_bs(k), :])
        vtr.append(t)
    vna = []
    for j in range(NB):
        t = consts.tile([128, S], FPR, tag=f"vnat{j}", name=f"vnat{j}")
        nc.sync.dma_start(out=t, in_=vnat_d[_bs(j), :])
        vna.append(t)
    for k in range(4):
        nc.sync.dma_start(out=xb[k], in_=xt_d[_bs(k), :])

    # --- mask constants (GpSimd/DVE, overlap the DMAs) ---
    eye = consts.tile([128, 128], FP, tag="eye")
    make_identity(nc, eye)
    triu = consts.tile([128, 128], FP, tag="triu")
    make_upper_triangular(nc, triu, val=1.0, diag=False)
    padcol = consts.tile([128, 1], FP, tag="padcol")
    nc.gpsimd.memset(padcol, 1.0)
    nc.gpsimd.affine_select(
        out=padcol, in_=padcol, compare_op=OP.is_ge, fill=0.0,
        base=-(NV - 4 * 128), pattern=[[0, 1]], channel_multiplier=1,
    )
    # (ACT, not gpsimd: gpsimd carries ~2us fixed dispatch cost per op and
    # the last eye2all slice gates Newton's first m2 computation)
    eye2all = consts.tile([128, NP], FP, tag="eye2all")
    for b in range(NB):
        nc.scalar.activation(eye2all[:, _bs(b)], eye,
                             mybir.ActivationFunctionType.Copy, scale=2.0)
    eye_bf = consts.tile([128, 128], BF, tag="eye_bf")
    nc.scalar.copy(eye_bf, eye)
    # --- G = V^T V in f32r, wide row strips (moving dim >= 256), fused
    # with the per-group rd/RT/X0 chains: group a (blocks 0-2) has its
    # whole Newton-entry chain emitted right after block 2's strip, so it
    # runs on DVE while the PE is still doing blocks 3-4 of G. ---
    chunks = {0: [(0, 512), (384, 640)], 1: [(128, 640)], 2: [(256, 640)],
              3: [(384, 640)], 4: [(384, 640)]}
    grow = []    # strictly-upper row strips, f32r (cols re-based)
    grow_w = [512, 384, 256, 128]
    for b in range(NB - 1):
        grow.append(consts.tile([128, grow_w[b]], FPR, tag=f"g{b}",
                                name=f"g{b}"))
    gda = consts.tile([128, WA], FP, tag="gda")
    gdb = consts.tile([128, WB], FP, tag="gdb")
    triua = consts.tile([128, WA], FP, tag="triua")
    for b in range(NGA):
        nc.gpsimd.tensor_copy(triua[:, _bs(b)], triu)
    triub = consts.tile([128, WB], FP, tag="triub")
    for b in range(NB - NGA):
        nc.gpsimd.tensor_copy(triub[:, _bs(b)], triu)
    rd_a = consts.tile([128, NGA], FP, tag="rd_a")
    rd_b = consts.tile([128, NB - NGA], FP, tag="rd_b")
    ri_a = consts.tile([128, NGA], FP, tag="ri_a")
    ri_b = consts.tile([128, NB - NGA], FP, tag="ri_b")
    rta = consts.tile([128, WA], BF, tag="rta")
    rtb = consts.tile([128, WB], BF, tag="rtb")
    xa = work.tile([128, WA], BF, tag="xa")
    xb2 = work.tile([128, WB], BF, tag="xb2")
    ca = work.tile([128, WA], BF, tag="ca")
    cb = work.tile([128, WB], BF, tag="cb")

    def rt_slice(b):
        return rta[:, _bs(b)] if b < NGA else rtb[:, _bs(b - NGA)]

    def emit_group_entry(grp):
        # (rd+eps)/2 [+1 pad], reciprocal, RT build, X0=C0=diag(1/rd).
        # The mask-mul is one whole-group DVE op; only the per-block
        # diag-add (per-partition scalar) stays per-block.
        rd, ri = (rd_a, ri_a) if grp == 0 else (rd_b, ri_b)
        blocks = range(NGA) if grp == 0 else range(NGA, NB)
        gdg = gda if grp == 0 else gdb
        rtg = rta if grp == 0 else rtb
        triug = triua if grp == 0 else triub
        wg = WA if grp == 0 else WB
        nc.vector.tensor_scalar(rd, rd, EPS, 0.5, OP.add, OP.mult)
        if grp == 1:
            nc.vector.tensor_add(rd[:, -1:], rd[:, -1:], padcol)
        nc.vector.reciprocal(ri, rd)
        rtm = work.tile([128, wg], FP, tag=f"rtm{grp}")
        nc.vector.tensor_mul(rtm, gdg, triug)
        for b in blocks:
            g = b if grp == 0 else b - NGA
            nc.vector.scalar_tensor_tensor(
                out=rtg[:, _bs(g)], in0=eye, scalar=rd[:, g:g + 1],
                in1=rtm[:, _bs(g)], op0=OP.mult, op1=OP.add)
            dst = xa[:, _bs(b)] if grp == 0 else xb2[:, _bs(b - NGA)]
            nc.scalar.activation(dst, eye, mybir.ActivationFunctionType.Copy,
                                 scale=ri[:, g:g + 1])
        if grp == 0:
            nc.scalar.copy(ca, xa)
        else:
            nc.scalar.copy(cb, xb2)

    gtag = [0]
    for b in range(NB):
        g_pss = []
        for (c0, c1) in chunks[b]:
            g_ps = psum.tile([128, c1 - c0], FP, tag=f"y{gtag[0] % 6}",
                             name=f"g{b}_{c0}")
            gtag[0] += 1
            for k in range(4):
                nc.tensor.matmul(g_ps, lhsT=vtr[k][:, _bs(b)],
                                 rhs=vtr[k][:, c0:c1],
                                 start=(k == 0), stop=(k == 3))
            g_pss.append((c0, g_ps))
        c0d, psd = g_pss[0] if b < 4 else g_pss[-1]
        gslice = (gda[:, _bs(b)] if b < NGA else gdb[:, _bs(b - NGA)])
        nc.scalar.copy(gslice, psd[:, b * 128 - c0d:(b + 1) * 128 - c0d])
        # rd terms for this block (DVE, overlaps later G matmuls)
        dt = work.tile([128, 128], FP, tag="dtmp")
        nc.vector.tensor_mul(dt, gslice, eye)
        rd, g = (rd_a, b) if b < NGA else (rd_b, b - NGA)
        nc.vector.reduce_sum(rd[:, g:g + 1], dt, axis=AX.X)
        # strictly-upper strip copies (DVE writes the f32r tiles)
        if b < 4:
            u0 = (b + 1) * 128
            copied_to = u0
            for (c0, g_ps) in g_pss:
                c1 = c0 + g_ps.shape[1]
                lo = max(copied_to, c0)
                if c1 <= lo:
                    continue
                nc.vector.tensor_copy(grow[b][:, lo - u0:c1 - u0],
                                      g_ps[:, lo - c0:c1 - c0])
                copied_to = c1
        if b == NGA - 1:
            emit_group_entry(0)
        if b == NB - 1:
            emit_group_entry(1)

    def goff(k, i):  # G[k-block, i-block] as lhsT, i > k
        return grow[k][:, (i - k - 1) * 128:(i - k) * 128]

    for it in range(NEWTON):
        last = (it == NEWTON - 1)
        m1a = psum.tile([128, WA], FP, tag="y0", name=f"m1a{it}")
        m1b = psum.tile([128, WB], FP, tag="y1", name=f"m1b{it}")
        for b in range(NGA):
            nc.tensor.matmul(m1a[:, _bs(b)], lhsT=rta[:, _bs(b)],
                             rhs=xa[:, _bs(b)], start=True, stop=True)
        for b in range(NB - NGA):
            nc.tensor.matmul(m1b[:, _bs(b)], lhsT=rtb[:, _bs(b)],
                             rhs=xb2[:, _bs(b)], start=True, stop=True)
        m2a = work.tile([128, WA], BF, tag="m2a")
        nc.vector.scalar_tensor_tensor(
            out=m2a, in0=m1a, scalar=-1.0, in1=eye2all[:, 0:WA],
            op0=OP.mult, op1=OP.add)
        m2b = work.tile([128, WB], BF, tag="m2b")
        nc.vector.scalar_tensor_tensor(
            out=m2b, in0=m1b, scalar=-1.0, in1=eye2all[:, WA:NP],
            op0=OP.mult, op1=OP.add)
        xna = psum.tile([128, WA], FP, tag="y2", name=f"xna{it}")
        cna = psum.tile([128, WA], FP, tag="y4", name=f"cna{it}")
        for b in range(NGA):
            nc.tensor.matmul(xna[:, _bs(b)], lhsT=ca[:, _bs(b)],
                             rhs=m2a[:, _bs(b)], start=True, stop=True)
            nc.tensor.matmul(cna[:, _bs(b)], lhsT=m2a[:, _bs(b)],
                             rhs=ca[:, _bs(b)], start=True, stop=True)
        xnb = psum.tile([128, WB], FP, tag="y3", name=f"xnb{it}")
        cnb = psum.tile([128, WB], FP, tag="y5", name=f"cnb{it}")
        for b in range(NB - NGA):
            nc.tensor.matmul(xnb[:, _bs(b)], lhsT=cb[:, _bs(b)],
                             rhs=m2b[:, _bs(b)], start=True, stop=True)
            nc.tensor.matmul(cnb[:, _bs(b)], lhsT=m2b[:, _bs(b)],
                             rhs=cb[:, _bs(b)], start=True, stop=True)
        if last:
            break
        xa = work.tile([128, WA], BF, tag="xa")
        nc.scalar.copy(xa, xna)
        ca = work.tile([128, WA], BF, tag="ca")
        nc.vector.tensor_copy(ca, cna)
        xb2 = work.tile([128, WB], BF, tag="xb2")
        nc.vector.tensor_copy(xb2, xnb)
        cb = work.tile([128, WB], BF, tag="cb")
        nc.scalar.copy(cb, cnb)

    # converged diagonal inverse (and its transpose) in f32r, copied
    # straight from the last iteration's PSUMs (the BIR verifier requires
    # f32r matmul operands to be PRODUCED as f32r — DVE-only)
    xfa = consts.tile([128, WA], FPR, tag="xfa")
    nc.vector.tensor_copy(xfa, xna)
    xfb = consts.tile([128, WB], FPR, tag="xfb")
    nc.vector.tensor_copy(xfb, xnb)
    cfa = consts.tile([128, WA], FPR, tag="cfa")
    nc.vector.tensor_copy(cfa, cna)
    cfb = consts.tile([128, WB], FPR, tag="cfb")
    nc.vector.tensor_copy(cfb, cnb)

    def xdiag(b):
        return xfa[:, _bs(b)] if b < NGA else xfb[:, _bs(b - NGA)]

    def cdiag(b):
        return cfa[:, _bs(b)] if b < NGA else cfb[:, _bs(b - NGA)]

    # bf16 copies of V for the (bf16) A-stage matmuls; ACT, off-path
    vna_bf = []
    for j in range(NB):
        t = consts.tile([128, S], BF, tag=f"vbf{j}", name=f"vbf{j}")
        nc.scalar.copy(t, vna[j])
        vna_bf.append(t)

    # zero-padded X rows for the wide back-substitution (cols j = 0..3).
    # Only the regions read before written need pre-zeroing: row k's cols
    # (k+1)*128..512, for k <= 2.  Zeroed via DVE multiply-by-zero
    # (gpsimd memset can't write f32r).
    xfull = []
    for k in range(NB):
        t = consts.tile([128, 512], FPR, tag=f"xf{k}", name=f"xf{k}")
        if k <= 2:
            z0 = (k + 1) * 128
            nc.vector.tensor_scalar_mul(t[:, z0:512], vna[0][:, 0:512 - z0],
                                        0.0)
        xfull.append(t)
    for k in range(4):
        nc.vector.tensor_copy(xfull[k][:, _bs(k)], xdiag(k))

    # --- back-substitution + progressive WT accumulation ---
    # step i: X_i,(0:i) = -X_ii * sum_{k<i} G_ki^T Xrow_k[0:i*128]
    # (rows are zero-padded above the diagonal, so one wide matmul per k).
    # After each row i is final, its WT terms  wt_ps[j] += X_ij^T vna_i
    # accumulate immediately (interleaved PSUM groups on separate banks).
    wt_ps = [psum.tile([128, S], FP, tag=f"y{j}", name=f"wtp{j}")
             for j in range(NB)]

    def emit_wt_terms(i):
        for j in range(i + 1):
            z = xdiag(i) if j == i else xfull[i][:, _bs(j)]
            nc.tensor.matmul(wt_ps[j], lhsT=z, rhs=vna[i],
                             start=(i == j), stop=(i == NB - 1),
                             skip_group_check=True)

    emit_wt_terms(0)
    for i in range(1, NB):
        w = i * 128
        acc_ps = psum.tile([128, w], FP, tag="y5", name=f"acc{i}")
        for k in range(i):
            nc.tensor.matmul(acc_ps, lhsT=goff(k, i), rhs=xfull[k][:, 0:w],
                             start=(k == 0), stop=(k == i - 1))
        nacc = work.tile([128, w], FPR, tag="nacc")
        nc.vector.tensor_scalar_mul(nacc, acc_ps, -1.0)
        xij_ps = psum.tile([128, w], FP, tag="y6", name=f"xij{i}")
        nc.tensor.matmul(xij_ps, lhsT=cdiag(i), rhs=nacc,
                         start=True, stop=True)
        nc.vector.tensor_copy(xfull[i][:, 0:w], xij_ps)
        emit_wt_terms(i)

    wt_sb = []
    for j in range(NB):
        wt = consts.tile([128, S], BF, tag=f"wt{j}", name=f"wt{j}")
        nc.scalar.copy(wt, wt_ps[j])
        wt_sb.append(wt)

    # --- A = I - WT^T vnat, cast to bf16 (4 tiles [128(s), 512(c)]) ---
    a_bf = []
    for st in range(4):
        a_ps = ptile([128, S], f"a{st}")
        for j in range(NB):
            nc.tensor.matmul(a_ps, lhsT=wt_sb[j][:, _bs(st)],
                             rhs=vna_bf[j],
                             start=(j == 0), stop=(j == NB - 1))
        a = consts.tile([128, S], BF, tag=f"a{st}", name=f"a{st}")
        nc.scalar.mul(a, a_ps, -1.0)
        nc.vector.tensor_add(a[:, _bs(st)], a[:, _bs(st)], eye_bf)
        a_bf.append(a)
    return a_bf


def build_program(trace_sim=False):
    nc = bass.Bass("TRN2")
    xt_d = nc.dram_tensor("xt", [S, BPC], BF, kind="ExternalInput")
    vt_d = nc.dram_tensor("vt", [S, NP], FPR, kind="ExternalInput")
    vnat_d = nc.dram_tensor("vnat", [NP, S], FPR, kind="ExternalInput")
    y_d = nc.dram_tensor("y", [S, BPC], BF, kind="ExternalOutput")

    with tile.TileContext(nc, trace_sim=trace_sim) as tc, ExitStack() as ctx:
        consts = ctx.enter_context(tc.tile_pool(name="consts", bufs=1))
        work = ctx.enter_context(tc.tile_pool(name="work", bufs=2))
        ypool = ctx.enter_context(tc.tile_pool(name="ypool", bufs=3))
        psum = ctx.enter_context(
            tc.tile_pool(name="psum", bufs=1, space="PSUM"))

        # x^T resident in SBUF (8 MB bf16)
        xb = [consts.tile([128, BPC], BF, tag=f"xb{k}", name=f"xb{k}")
              for k in range(4)]

        a_bf = _emit_prologue(nc, vt_d, vnat_d, xt_d, xb, consts, work, psum)

        # --- main loop: y^T[c,r] = sum_k A[k-strip, c-strip]^T x^T[k, r] ---
        nmm = 0
        for c in range(4):
            for rg in range(BPC // RG):
                yt = ypool.tile([128, RG], BF, tag="yt")
                for r in range(RG // RW):
                    y_ps = psum.tile([128, RW], FP, tag=f"y{nmm % 7}")
                    nmm += 1
                    r0 = rg * RG + r * RW
                    for k in range(4):
                        nc.tensor.matmul(
                            y_ps,
                            lhsT=a_bf[k][:, _bs(c)],
                            rhs=xb[k][:, r0:r0 + RW],
                            start=(k == 0), stop=(k == 3))
                    if r % 2 == 0:
                        nc.scalar.copy(yt[:, r * RW:(r + 1) * RW], y_ps)
                    else:
                        nc.vector.tensor_copy(yt[:, r * RW:(r + 1) * RW], y_ps)
                nc.sync.dma_start(
                    out=y_d[_bs(c), rg * RG:(rg + 1) * RG], in_=yt)
    _split_excess_waits(nc)
    return nc


_NC_CACHE = {}


def _get_nc():
    if "nc" not in _NC_CACHE:
        _NC_CACHE["nc"] = build_program()
    return _NC_CACHE["nc"]


def prepare_in_maps(x, vectors):
    x = np.asarray(x, dtype=np.float32)
    v = np.asarray(vectors, dtype=np.float32)[..., 0]  # [514, 512]
    vnat = np.zeros((NP, S), np.float32)
    vnat[:NV] = v
    vt = np.ascontiguousarray(vnat.T)                  # [512, 640]
    xbf = x.astype(ml_dtypes.bfloat16)                 # [65536, 512] bf16
    xt = np.ascontiguousarray(xbf.T)                   # [512, 65536] bf16
    in_maps = []
    for c in range(NCORES):
        in_maps.append({
            "xt": np.ascontiguousarray(xt[:, c * BPC:(c + 1) * BPC]),
            "vt": vt,
            "vnat": vnat,
        })
    return in_maps


def finish_output(res):
    yt = np.concatenate([r["y"] for r in res.results], axis=1)  # [512, 65536]
    y = yt.T.astype(np.float32)                                 # [65536, 512]
    return np.ascontiguousarray(y)


def kernel(x, vectors):
    nc = _get_nc()
    in_maps = prepare_in_maps(x, vectors)
    res = run_bass_kernel_spmd(nc, in_maps, list(range(NCORES)))
    return finish_output(res)


if __name__ == "__main__":
    rng = np.random.default_rng(0)
    x = rng.standard_normal((B, S)).astype(np.float32)
    v = rng.standard_normal((NV, S, 1)).astype(np.float32)
    v /= np.linalg.norm(v, axis=1, keepdims=True)
    y = kernel(x, v)
    print("y", y.shape, y.dtype, float(np.abs(y).max()))
